# revision 64
# baseline (speedup 1.0000x reference)
"""Trainium2 Bass kernel for capsule attention-routing.

Reference computation (per pixel; 4096 independent problems of shape
[I=32 in-caps, N=32 out-caps, J=16 caps-dim]):
    v[n,j]   = sum_i u[i,n,j]
    cp[i,n]  = sum_j u[i,n,j] * v[n,j] / 4
    c[i,n]   = softmax_n(cp)[i,n] + b[i,n]
    s[n,j]   = sum_i u[i,n,j] * c[i,n]
    out[n,j] = (1 - exp(-|s|_j)) * s[n,j] / |s|_j

Sharding: data-parallel over (batch, h-half): 8 cores x 512 pixels.

Per-core strategy (dual layout, u streamed twice from HBM as fp16):
  L1 (j-major): partitions (j*8+il), free (ib, n, p64)  [il=i%8, i=ib*8+il]
     - v-pass: PE contracts il (+PSUM accum over ib), broadcast over rows
     - c-mult: DVE 2x-mode fp16 w = u1 * v
     - c-red : PE contracts j via banded 0.25-delta weights ->
               cp[(g*8+il) parts, (ib,n,p4)]  [g = pixel>>2]
  softmax over n on the small cp tile (Act exp f32, Pool z-reduce,
  DVE reciprocal+mult -> c_sb fp16)
  L2 (i-major): partitions (g*8+il), free (ib, n, j, p4)
     - s-mult: DVE 2x-mode m2 = u2 * broadcast_j(c_sb)  (no PE broadcast,
               no PSUM->SBUF copies: c broadcasts via a stride-0 free dim)
     - s-red : PE contracts il within g strips (+accum over ib) ->
               s[(g,x8-replicated) parts, (n8,j,p4)] in 4 nq PSUM banks
  squash: Act square, Pool j-reduce, r = exp(.5*ln(n2)) / rn = exp(-.5*ln n2)
  (single act table: no LoadActFuncSet flips), Pool final multiply.
Softmax runs without max-subtraction (|cp| <~ 45 is safe in fp32 exp).
EPS=1e-20 is negligible: 1-1/(exp(r)+eps) == 1-exp(-r), 1/(r+eps) == 1/r.
"""

import numpy as np
from contextlib import ExitStack

import concourse.bass as bass
import concourse.bacc as bacc
import concourse.tile as tile
import concourse.mybir as mybir
from concourse.bass_utils import run_bass_kernel_spmd

dt = mybir.dt
AF = mybir.ActivationFunctionType
OP = mybir.AluOpType

B, I, N, J, H, W = 4, 32, 32, 16, 32, 32
HW = H * W
NCORES = 8
PIX = B * HW // NCORES      # 512 pixels per core
BLK = 64                    # pixels per block
NBLK = PIX // BLK           # 8
NG = 16                     # pixel groups of 4 per block (g = pixel>>2)
P4 = 4
SCALE = 0.25                # 1/sqrt(16)

f32, bf16, f16 = dt.float32, dt.bfloat16, dt.float16


def _build_weight_arrays():
    il_of = np.arange(128) % 8          # L1 partition -> il is p%8? no: p=(j,il)
    # L1 partitions: p = j*8 + il  -> j = p//8, il = p%8
    j_of = np.arange(128) // 8
    il1 = np.arange(128) % 8

    # v-pass: out[(j2,il2)] = sum_il u[(j,il)] for j==j2 (broadcast over il2)
    wv = np.zeros((128, 128), np.float32)
    for p_in in range(128):
        for p_out in range(128):
            if j_of[p_in] == j_of[p_out]:
                wv[p_in, p_out] = 1.0

    # c-red band: window at offset off(g)=2*(120 - g*8) bytes gives the
    # [128,128] weight mapping (j,il) -> out partition (g*8+il), scaled 0.25.
    # band[(j,il), c] = 0.25 iff c == 120 + il
    wc_band = np.zeros((128, 248), np.float32)
    for p_in in range(128):
        wc_band[p_in, 120 + il1[p_in]] = SCALE

    # s-red band: window at offset off(jq)=2*(6 - jq*2) bytes maps L2
    # partitions (g,il) -> out partition (g*8 + jq*2 + r), r=0,1 replicas.
    # band[(g,il), c] = 1 iff c in (g*8+6, g*8+7)
    ws_band = np.zeros((128, 134), np.float32)
    g_of = np.arange(128) // 8
    for p_in in range(128):
        ws_band[p_in, g_of[p_in] * 8 + 6] = 1.0
        ws_band[p_in, g_of[p_in] * 8 + 7] = 1.0

    # n2: contract the 8 rows of each g strip (each real value appears
    # twice via the r2 replicas -> 0.5)
    wn = np.zeros((128, 128), np.float32)
    for p_in in range(128):
        for p_out in range(128):
            if p_in // 8 == p_out // 8:
                wn[p_in, p_out] = 0.5

    return {"wv": wv, "wc_band": wc_band, "ws_band": ws_band, "wn": wn}


def _b_tile_array(b_np):
    # bt[(g*8+il), (ib, n, p4)] = b[ib*8+il, n]
    bt = np.zeros((128, 4 * N * P4), np.float32)
    bsl = np.asarray(b_np).reshape(I, N)
    for g in range(NG):
        for il in range(8):
            row = g * 8 + il
            for ib in range(4):
                for n in range(N):
                    bt[row, (ib * N + n) * P4 : (ib * N + n + 1) * P4] = bsl[
                        ib * 8 + il, n
                    ]
    return bt


def _emit(ctx: ExitStack, tc: tile.TileContext, aps: dict, with_b: bool):
    nc = tc.nc
    u1_d, u2_d, o_d = aps["u1"], aps["u2"], aps["out"]

    # constants
    pconst = ctx.enter_context(tc.tile_pool(name="const", bufs=1))
    wv_t = pconst.tile([128, 128], f16, tag="wv")
    wcb_t = pconst.tile([128, 248], f16, tag="wcb")
    ws_t = pconst.tile([128, 134], f16, tag="ws")
    wn_t = pconst.tile([128, 128], f16, tag="wn")
    bt_t = None
    if with_b:
        bt_t = pconst.tile([128, 4 * N * P4], f32, tag="bt")
        nc.sync.dma_start(bt_t[:], aps["bt"])
    eps_t = pconst.tile([128, 1], f32, tag="eps")
    nc.gpsimd.memset(eps_t[:], 1e-30)

    # pools
    pu1 = ctx.enter_context(tc.tile_pool(name="u1", bufs=3))
    pu2 = ctx.enter_context(tc.tile_pool(name="u2", bufs=2))
    pw1 = ctx.enter_context(tc.tile_pool(name="w1", bufs=2))
    pm2 = ctx.enter_context(tc.tile_pool(name="m2", bufs=3))
    pvsb = ctx.enter_context(tc.tile_pool(name="vsb", bufs=3))
    pce = ctx.enter_context(tc.tile_pool(name="ce", bufs=2))
    pcsb = ctx.enter_context(tc.tile_pool(name="csb", bufs=2))
    psq = ctx.enter_context(tc.tile_pool(name="sq", bufs=2))
    pout = ctx.enter_context(tc.tile_pool(name="out", bufs=2))

    pcsb2 = ctx.enter_context(tc.tile_pool(name="csb2", bufs=3))

    pvps = ctx.enter_context(tc.tile_pool(name="vps", bufs=3, space="PSUM"))
    pcps = ctx.enter_context(tc.tile_pool(name="cps", bufs=2, space="PSUM"))
    psps = ctx.enter_context(tc.tile_pool(name="sps", bufs=2, space="PSUM"))
    pnps = ctx.enter_context(tc.tile_pool(name="nps", bufs=1, space="PSUM"))

    loads1 = {}
    loads2 = {}

    def load1(blk):
        # u1 split into 4 st-chunks: [(j,il), (ib, n, p16)] each
        ts = []
        for st in range(4):
            u1 = pu1.tile([128, 4 * N * 16], f16, tag=f"u1st{st}")
            nc.sync.dma_start(u1[:], u1_d[blk, st])
            ts.append(u1)
        loads1[blk] = ts

    def load2(blk):
        # u2 split into 4 ib-chunks: [(g,il), (j, n, p4)] each
        ts = []
        for ib in range(4):
            u2 = pu2.tile([128, J * N * P4], f16, tag=f"u2ib{ib}")
            nc.sync.dma_start(u2[:], u2_d[blk, ib])
            ts.append(u2)
        loads2[blk] = ts

    def s1(blk):
        """v-pass (PE) -> v-copies (Act); prefetch next u1."""
        if blk + 1 < NBLK:
            load1(blk + 1)
        u1s = loads1.pop(blk)

        # ---- v-pass (PE): v[(j,il-bcast),(st,n,p16)] = sum_i u1 ----
        v_sb = pvsb.tile([128, N * BLK], f16, tag="vsb")
        v_sb_v = v_sb[:].rearrange("P (st f) -> P st f", st=4)
        for st in range(4):
            u1_v = u1s[st][:].rearrange("P (ib f) -> P ib f", ib=4)
            v_ps = pvps.tile([128, 512], f32, tag="vps")
            for ib in range(4):
                nc.tensor.matmul(
                    v_ps[:],
                    wv_t[:],
                    u1_v[:, ib],
                    start=(ib == 0),
                    stop=(ib == 3),
                )
            nc.scalar.copy(v_sb_v[:, st], v_ps[:])
        return u1s, v_sb

    def s2(blk, u1s, v_sb):
        """c-mult (DVE) -> c-red (PE) -> softmax; prefetch u2."""
        load2(blk)
        v_sb_v = v_sb[:].rearrange("P (st f) -> P st f", st=4)

        # ---- c-mult (2x): w1 = u1 * bcast_ib(v); st0 on Pool (it has the
        # most lead time), st1-3 on DVE ----
        w1s = []
        for st in range(4):
            u1_v = u1s[st][:].rearrange("P (ib f) -> P ib f", ib=4)
            w1 = pw1.tile([128, 4 * N * 16], f16, tag=f"w1st{st}")
            vb = (
                v_sb_v[:, st]
                .rearrange("P (o f) -> P o f", o=1)
                .broadcast_to([128, 4, N * 16])
            )
            eng = nc.gpsimd if st == 0 else nc.vector
            eng.tensor_tensor(
                w1[:].rearrange("P (ib f) -> P ib f", ib=4), u1_v, vb,
                op=OP.mult,
            )
            w1s.append(w1)

        # ---- c-red (PE): cp[(g,il), (ib,n,p4)] = 0.25*sum_j w1 ----
        # st0's g-group last: its w1 comes from the slower Pool engine
        cp = pcps.tile([128, 4 * N * P4], f32, tag="cp")
        cp_v = cp[:].rearrange("P (ib n p) -> P ib n p", ib=4, p=P4)
        g_order = list(range(4, NG)) + list(range(4))
        for i, g in enumerate(g_order):
            st, gl = g // 4, g % 4
            off = 120 - g * 8
            w1_v = w1s[st][:].rearrange(
                "P (ib n p) -> P ib n p", ib=4, p=16
            )
            nc.tensor.matmul(
                cp_v,
                wcb_t[:, off : off + 128],
                w1_v[:, :, :, gl * P4 : (gl + 1) * P4],
                start=(i == 0),
                stop=(i == NG - 1),
                skip_group_check=True,
            )

        # ---- softmax over n (no max-subtraction) ----
        c_e = pce.tile([128, 4 * N * P4], f32, tag="ce")
        nc.scalar.activation(c_e[:], cp[:], AF.Exp)
        c_e_v = c_e[:].rearrange("P (ib n p) -> P ib n p", ib=4, p=P4)
        z = pcsb.tile([128, 4 * P4], f32, tag="z")
        z_v = z[:].rearrange("P (ib p) -> P ib p", ib=4)
        for ib in range(4):
            nc.vector.tensor_reduce(
                z_v[:, ib],
                c_e_v[:, ib].rearrange("P n p -> P p n"),
                axis=mybir.AxisListType.X,
                op=OP.add,
            )
        rz = pcsb.tile([128, 4 * P4], f32, tag="rz")
        nc.vector.reciprocal(rz[:], z[:])
        rz_v = rz[:].rearrange("P (ib p) -> P ib p", ib=4)
        c_sb = pcsb2.tile([128, 4 * N * P4], f16, tag="csb")
        c_sb_v = c_sb[:].rearrange("P (ib n p) -> P ib n p", ib=4, p=P4)
        for ib in range(4):
            rz_b = (
                rz_v[:, ib]
                .rearrange("P (o p) -> P o p", o=1)
                .broadcast_to([128, N, P4])
            )
            if with_b:
                c_f = pcsb.tile([128, N * P4], f32, tag="cf")
                nc.gpsimd.tensor_tensor(
                    c_f[:].rearrange("P (n p) -> P n p", p=P4),
                    c_e_v[:, ib],
                    rz_b,
                    op=OP.mult,
                )
                bt_v = bt_t[:].rearrange("P (ib n p) -> P ib n p", ib=4, p=P4)
                nc.gpsimd.tensor_tensor(
                    c_sb_v[:, ib], c_f[:].rearrange("P (n p) -> P n p", p=P4),
                    bt_v[:, ib], op=OP.add,
                )
            else:
                nc.gpsimd.tensor_tensor(
                    c_sb_v[:, ib], c_e_v[:, ib], rz_b, op=OP.mult
                )
        return c_sb

    def s3(blk, c_sb):
        """m2 -> s-red -> squash -> store."""
        u2s = loads2.pop(blk)
        c_sb_v2 = c_sb[:].rearrange("P (ib f) -> P ib f", ib=4)  # f = (n p)
        # s_all[(g, jq, r2) parts, (jl4, n32, p4)]; j = jq*4 + jl
        s_all = psps.tile([128, 4 * N * P4], f32, tag="sall")
        s_all_v = s_all[:].rearrange("P (jl f) -> P jl f", jl=4)
        for ib in range(4):
            u2_v = u2s[ib][:].rearrange("P (j f) -> P j f", j=J)
            cb = (
                c_sb_v2[:, ib]
                .rearrange("P (o f) -> P o f", o=1)
                .broadcast_to([128, 4, N * P4])
            )
            for jq in range(4):
                m2 = pm2.tile([128, 4 * N * P4], f16, tag=f"m2q{jq}")
                m2_v = m2[:].rearrange("P (j f) -> P j f", j=4)
                nc.vector.tensor_tensor(
                    m2_v, u2_v[:, jq * 4 : (jq + 1) * 4], cb, op=OP.mult
                )
                off = 6 - jq * 2
                nc.tensor.matmul(
                    s_all_v,
                    ws_t[:, off : off + 128],
                    m2_v,
                    start=(ib == 0 and jq == 0),
                    stop=(ib == 3 and jq == 3),
                    skip_group_check=True,
                )

        # ---- squash ----
        # ssq = s^2 (bf16 keeps fp32 range; fp16 would flush subnormals)
        ssq = psq.tile([128, 4 * N * P4], bf16, tag="ssq")
        nc.scalar.activation(ssq[:], s_all[:], AF.Square)
        ssq_v = ssq[:].rearrange("P (jl f) -> P jl f", jl=4)
        t1 = psq.tile([128, 2 * N * P4], bf16, tag="t1")
        t1_v = t1[:].rearrange("P (jl f) -> P jl f", jl=2)
        nc.gpsimd.tensor_tensor(t1_v, ssq_v[:, 0:2], ssq_v[:, 2:4], op=OP.add)
        ssq_l = psq.tile([128, N * P4], bf16, tag="ssql")
        nc.gpsimd.tensor_tensor(ssq_l[:], t1_v[:, 0], t1_v[:, 1], op=OP.add)
        # n2[(g,x8), (n,p4)] = sum_j s^2 via PE partition contraction
        n2 = pnps.tile([128, 4 * 8 * P4], f32, tag="n2")
        nc.tensor.matmul(n2[:], wn_t[:], ssq_l[:], start=True, stop=True)
        # ln(n2 + 1e-30): the bias keeps ln finite at n2==0 (out ~0 there)
        lnn = psq.tile([128, 4 * 8 * P4], f32, tag="lnn")
        nc.scalar.activation(lnn[:], n2[:], AF.Ln, bias=eps_t[:])
        # r = exp(.5 ln n2) = |s|; rn = exp(-.5 ln n2) = 1/|s|
        r_t = psq.tile([128, 4 * 8 * P4], f32, tag="r")
        nc.scalar.activation(r_t[:], lnn[:], AF.Exp, scale=0.5)
        rn_t = psq.tile([128, 4 * 8 * P4], f32, tag="rn")
        nc.scalar.activation(rn_t[:], lnn[:], AF.Exp, scale=-0.5)
        en_t = psq.tile([128, 4 * 8 * P4], f32, tag="en")
        nc.scalar.activation(en_t[:], r_t[:], AF.Exp, scale=-1.0)
        g_t = psq.tile([128, N * P4], f32, tag="g")
        nc.vector.scalar_tensor_tensor(
            g_t[:], en_t[:], 1.0, rn_t[:], op0=OP.subtract, op1=OP.mult
        )  # g = (en - 1) / r
        g_b = (
            g_t[:]
            .rearrange("P (o f) -> P o f", o=1)
            .broadcast_to([128, 4, N * P4])
        )

        # out_dev = s * g = -out_true; the host decode negates (keeps the
        # final multiply off DVE: Act copies s out of PSUM, Pool multiplies)
        s_sb = psq.tile([128, 4 * N * P4], f16, tag="ssb")
        nc.scalar.copy(s_sb[:], s_all[:])
        outt = pout.tile([128, 4 * N * P4], f16, tag="outt")
        nc.gpsimd.tensor_tensor(
            outt[:].rearrange("P (jl f) -> P jl f", jl=4),
            s_sb[:].rearrange("P (jl f) -> P jl f", jl=4),
            g_b,
            op=OP.mult,
        )

        # only the r=0 replicas carry data: 64 partitions, stride 2
        nc.sync.dma_start(o_d[blk], outt[::2, :])

    # 3-stage software pipeline: emit s1(k), s2(k-1), s3(k-2) per iteration
    # so each in-order engine queue sees work in expected-ready order (the
    # PE queue in particular becomes v(k), c-red(k-1), s-red(k-2), each of
    # whose inputs is already in flight — PE stays continuously busy and at
    # full p-state).
    p1, p2 = {}, {}
    # startup: first u1 chunk, then wv (both needed by the first v matmul),
    # then the rest — each dma occupies HWDGE ~625ns, so order matters
    ts0 = []
    for st in range(4):
        u1_first = pu1.tile([128, 4 * N * 16], f16, tag=f"u1st{st}")
        ts0.append(u1_first)
    nc.sync.dma_start(ts0[0][:], u1_d[0, 0])
    nc.sync.dma_start(wv_t[:], aps["wv"])
    for st in range(1, 4):
        nc.sync.dma_start(ts0[st][:], u1_d[0, st])
    nc.sync.dma_start(wcb_t[:], aps["wc_band"])
    nc.sync.dma_start(ws_t[:], aps["ws_band"])
    nc.sync.dma_start(wn_t[:], aps["wn"])
    loads1[0] = ts0
    for it in range(NBLK + 2):
        if it < NBLK:
            p1[it] = s1(it)
        if 1 <= it <= NBLK:
            p2[it - 1] = s2(it - 1, *p1.pop(it - 1))
        if it >= 2:
            s3(it - 2, p2.pop(it - 2))


def round_f16(x):
    return x.astype(np.float16)


def encode_u1(shard):
    """[I, N, J, pix] -> [blk, st4, (j,il)=128, (ib,n,p16)] fp16."""
    a = shard.reshape(4, 8, N, J, NBLK, 4, 16)     # ib, il, n, j, blk, st, p16
    # -> blk, st, j, il, ib, n, p16
    return np.ascontiguousarray(
        a.transpose(4, 5, 3, 1, 0, 2, 6)
    ).astype(np.float16)


def encode_u2(shard):
    """[I, N, J, pix] -> [blk, ib4, (g,il)=128, (j,n,p4)] fp16."""
    a = shard.reshape(4, 8, N, J, NBLK, NG, P4)       # ib, il, n, j, blk, g, p4
    # -> blk, ib, g, il, j, n, p4
    return np.ascontiguousarray(
        a.transpose(4, 0, 5, 1, 3, 2, 6)
    ).astype(np.float16)


def decode_out(arr):
    """[blk, 64=(g,jq), (jl4,n32,p4)] fp16 -> [N, J, pix] f32.

    j = jq*4+jl; pixel = blk*64 + g*4 + p
    """
    a = arr.astype(np.float32).reshape(NBLK, NG, 4, 4, N, P4)
    # dims: blk, g, jq, jl, n, p -> n, (jq,jl), (blk,g,p)
    # negated: the device computes s*(en-1)/r = -out
    return -np.ascontiguousarray(a.transpose(4, 2, 3, 0, 1, 5)).reshape(
        N, J, PIX
    )


_CACHE = {}


def _patch_act_tables():
    """Keep only natural_log_exp_and_others (Copy/Exp/Ln/Square): every
    function this kernel uses lives in one table, so exactly ONE
    LoadActFuncSet is emitted. Other set entries are kept (emptied) to
    preserve act_func_set_id indices."""
    if getattr(bacc, "_ant_act_tables_patched", False):
        return
    real = bacc.get_activation_tables

    def patched(module_arch):
        tabs = real(module_arch)
        keep = {"natural_log_exp_and_others"}
        return {
            name: (fns if name in keep else set()) for name, fns in tabs.items()
        }

    bacc.get_activation_tables = patched
    bacc._ant_act_tables_patched = True


def _get_program(with_b=False):
    key = with_b
    if key in _CACHE:
        return _CACHE[key]
    _patch_act_tables()
    nc = bacc.Bacc("TRN2", target_bir_lowering=False, debug=False)
    aps = {}
    aps["u1"] = nc.dram_tensor(
        "u1", [NBLK, 4, 128, 4 * N * 16], f16, kind="ExternalInput"
    ).ap()
    aps["u2"] = nc.dram_tensor(
        "u2", [NBLK, 4, 128, J * N * P4], f16, kind="ExternalInput"
    ).ap()
    wts = _build_weight_arrays()
    aps["wv"] = nc.dram_tensor("wv", [128, 128], f16, kind="ExternalInput").ap()
    aps["wc_band"] = nc.dram_tensor(
        "wc_band", [128, 248], f16, kind="ExternalInput"
    ).ap()
    aps["ws_band"] = nc.dram_tensor(
        "ws_band", [128, 134], f16, kind="ExternalInput"
    ).ap()
    aps["wn"] = nc.dram_tensor("wn", [128, 128], f16, kind="ExternalInput").ap()
    if with_b:
        aps["bt"] = nc.dram_tensor(
            "bt", [128, 4 * N * P4], f32, kind="ExternalInput"
        ).ap()
    aps["out"] = nc.dram_tensor(
        "out", [NBLK, 64, 4 * N * P4], f16, kind="ExternalOutput"
    ).ap()

    with tile.TileContext(nc) as tc:
        with ExitStack() as ctx:
            _emit(ctx, tc, aps, with_b)
    nc.compile()

    _CACHE[key] = (nc, wts)
    return _CACHE[key]


def kernel(u: np.ndarray, b: np.ndarray) -> np.ndarray:
    u = np.asarray(u, dtype=np.float32)
    b = np.asarray(b, dtype=np.float32)
    with_b = bool(np.any(b))
    nc, wts = _get_program(with_b=with_b)

    base = {
        "wv": wts["wv"].astype(np.float16),
        "wc_band": wts["wc_band"].astype(np.float16),
        "ws_band": wts["ws_band"].astype(np.float16),
        "wn": wts["wn"].astype(np.float16),
    }
    if with_b:
        base["bt"] = _b_tile_array(b)
    in_maps = []
    for c in range(NCORES):
        bb = c // 2
        h0 = 16 * (c % 2)
        shard = u[bb, :, :, :, h0 : h0 + 16, :].reshape(I, N, J, PIX)
        m = dict(base)
        m["u1"] = encode_u1(shard)
        m["u2"] = encode_u2(shard)
        in_maps.append(m)

    res = run_bass_kernel_spmd(nc, in_maps, core_ids=list(range(NCORES)))
    out = np.zeros((B, N, J, H, W), np.float32)
    for c in range(NCORES):
        bb = c // 2
        h0 = 16 * (c % 2)
        out[bb, :, :, h0 : h0 + 16, :] = decode_out(res.results[c]["out"]).reshape(
            N, J, 16, W
        )
    return out


# revision 67
# speedup vs baseline: 1.0021x; 1.0021x over previous
"""Trainium2 Bass kernel for capsule attention-routing.

Reference computation (per pixel; 4096 independent problems of shape
[I=32 in-caps, N=32 out-caps, J=16 caps-dim]):
    v[n,j]   = sum_i u[i,n,j]
    cp[i,n]  = sum_j u[i,n,j] * v[n,j] / 4
    c[i,n]   = softmax_n(cp)[i,n] + b[i,n]
    s[n,j]   = sum_i u[i,n,j] * c[i,n]
    out[n,j] = (1 - exp(-|s|_j)) * s[n,j] / |s|_j

Sharding: data-parallel over (batch, h-half): 8 cores x 512 pixels.

Per-core strategy (dual layout, u streamed twice from HBM as fp16):
  L1 (j-major): partitions (j*8+il), free (ib, n, p64)  [il=i%8, i=ib*8+il]
     - v-pass: PE contracts il (+PSUM accum over ib), broadcast over rows
     - c-mult: DVE 2x-mode fp16 w = u1 * v
     - c-red : PE contracts j via banded 0.25-delta weights ->
               cp[(g*8+il) parts, (ib,n,p4)]  [g = pixel>>2]
  softmax over n on the small cp tile (Act exp f32, Pool z-reduce,
  DVE reciprocal+mult -> c_sb fp16)
  L2 (i-major): partitions (g*8+il), free (ib, n, j, p4)
     - s-mult: DVE 2x-mode m2 = u2 * broadcast_j(c_sb)  (no PE broadcast,
               no PSUM->SBUF copies: c broadcasts via a stride-0 free dim)
     - s-red : PE contracts il within g strips (+accum over ib) ->
               s[(g,x8-replicated) parts, (n8,j,p4)] in 4 nq PSUM banks
  squash: Act square, Pool j-reduce, r = exp(.5*ln(n2)) / rn = exp(-.5*ln n2)
  (single act table: no LoadActFuncSet flips), Pool final multiply.
Softmax runs without max-subtraction (|cp| <~ 45 is safe in fp32 exp).
EPS=1e-20 is negligible: 1-1/(exp(r)+eps) == 1-exp(-r), 1/(r+eps) == 1/r.
"""

import numpy as np
from contextlib import ExitStack

import concourse.bass as bass
import concourse.bacc as bacc
import concourse.tile as tile
import concourse.mybir as mybir
from concourse.bass_utils import run_bass_kernel_spmd

dt = mybir.dt
AF = mybir.ActivationFunctionType
OP = mybir.AluOpType

B, I, N, J, H, W = 4, 32, 32, 16, 32, 32
HW = H * W
NCORES = 8
PIX = B * HW // NCORES      # 512 pixels per core
BLK = 64                    # pixels per block
NBLK = PIX // BLK           # 8
NG = 16                     # pixel groups of 4 per block (g = pixel>>2)
P4 = 4
SCALE = 0.25                # 1/sqrt(16)

f32, bf16, f16 = dt.float32, dt.bfloat16, dt.float16


def _build_weight_arrays():
    il_of = np.arange(128) % 8          # L1 partition -> il is p%8? no: p=(j,il)
    # L1 partitions: p = j*8 + il  -> j = p//8, il = p%8
    j_of = np.arange(128) // 8
    il1 = np.arange(128) % 8

    # v-pass: out[(j2,il2)] = sum_il u[(j,il)] for j==j2 (broadcast over il2)
    wv = np.zeros((128, 128), np.float32)
    for p_in in range(128):
        for p_out in range(128):
            if j_of[p_in] == j_of[p_out]:
                wv[p_in, p_out] = 1.0

    # c-red band: window at offset off(g)=2*(120 - g*8) bytes gives the
    # [128,128] weight mapping (j,il) -> out partition (g*8+il), scaled 0.25.
    # band[(j,il), c] = 0.25 iff c == 120 + il
    wc_band = np.zeros((128, 248), np.float32)
    for p_in in range(128):
        wc_band[p_in, 120 + il1[p_in]] = SCALE

    # s-red band: window at offset off(jq)=2*(6 - jq*2) bytes maps L2
    # partitions (g,il) -> out partition (g*8 + jq*2 + r), r=0,1 replicas.
    # band[(g,il), c] = 1 iff c in (g*8+6, g*8+7)
    ws_band = np.zeros((128, 134), np.float32)
    g_of = np.arange(128) // 8
    for p_in in range(128):
        ws_band[p_in, g_of[p_in] * 8 + 6] = 1.0
        ws_band[p_in, g_of[p_in] * 8 + 7] = 1.0

    # n2: contract the 8 rows of each g strip (each real value appears
    # twice via the r2 replicas -> 0.5)
    wn = np.zeros((128, 128), np.float32)
    for p_in in range(128):
        for p_out in range(128):
            if p_in // 8 == p_out // 8:
                wn[p_in, p_out] = 0.5

    return {"wv": wv, "wc_band": wc_band, "ws_band": ws_band, "wn": wn}


def _b_tile_array(b_np):
    # bt[(g*8+il), (ib, n, p4)] = b[ib*8+il, n]
    bt = np.zeros((128, 4 * N * P4), np.float32)
    bsl = np.asarray(b_np).reshape(I, N)
    for g in range(NG):
        for il in range(8):
            row = g * 8 + il
            for ib in range(4):
                for n in range(N):
                    bt[row, (ib * N + n) * P4 : (ib * N + n + 1) * P4] = bsl[
                        ib * 8 + il, n
                    ]
    return bt


def _emit(ctx: ExitStack, tc: tile.TileContext, aps: dict, with_b: bool):
    nc = tc.nc
    u1_d, u2_d, o_d = aps["u1"], aps["u2"], aps["out"]

    # constants
    pconst = ctx.enter_context(tc.tile_pool(name="const", bufs=1))
    wv_t = pconst.tile([128, 128], f16, tag="wv")
    wcb_t = pconst.tile([128, 248], f16, tag="wcb")
    ws_t = pconst.tile([128, 134], f16, tag="ws")
    wn_t = pconst.tile([128, 128], f16, tag="wn")
    bt_t = None
    if with_b:
        bt_t = pconst.tile([128, 4 * N * P4], f32, tag="bt")
        nc.sync.dma_start(bt_t[:], aps["bt"])
    eps_t = pconst.tile([128, 1], f32, tag="eps")
    nc.gpsimd.memset(eps_t[:], 1e-30)

    # pools
    pu1 = ctx.enter_context(tc.tile_pool(name="u1", bufs=3))
    pu2 = ctx.enter_context(tc.tile_pool(name="u2", bufs=2))
    pw1 = ctx.enter_context(tc.tile_pool(name="w1", bufs=2))
    pm2 = ctx.enter_context(tc.tile_pool(name="m2", bufs=3))
    pvsb = ctx.enter_context(tc.tile_pool(name="vsb", bufs=3))
    pce = ctx.enter_context(tc.tile_pool(name="ce", bufs=2))
    pcsb = ctx.enter_context(tc.tile_pool(name="csb", bufs=2))
    psq = ctx.enter_context(tc.tile_pool(name="sq", bufs=2))
    pout = ctx.enter_context(tc.tile_pool(name="out", bufs=2))

    pcsb2 = ctx.enter_context(tc.tile_pool(name="csb2", bufs=3))

    pvps = ctx.enter_context(tc.tile_pool(name="vps", bufs=3, space="PSUM"))
    pcps = ctx.enter_context(tc.tile_pool(name="cps", bufs=2, space="PSUM"))
    psps = ctx.enter_context(tc.tile_pool(name="sps", bufs=2, space="PSUM"))
    pnps = ctx.enter_context(tc.tile_pool(name="nps", bufs=1, space="PSUM"))

    loads1 = {}
    loads2 = {}

    def load1(blk):
        # u1 split into 4 st-chunks: [(j,il), (ib, n, p16)] each
        ts = []
        for st in range(4):
            u1 = pu1.tile([128, 4 * N * 16], f16, tag=f"u1st{st}")
            nc.sync.dma_start(u1[:], u1_d[blk, st])
            ts.append(u1)
        loads1[blk] = ts

    def load2(blk):
        # u2 split into 4 ib-chunks: [(g,il), (j, n, p4)] each
        ts = []
        for ib in range(4):
            u2 = pu2.tile([128, J * N * P4], f16, tag=f"u2ib{ib}")
            nc.sync.dma_start(u2[:], u2_d[blk, ib])
            ts.append(u2)
        loads2[blk] = ts

    def s1(blk):
        """v-pass (PE) -> v-copies (Act); prefetch next u1."""
        if blk + 1 < NBLK:
            load1(blk + 1)
        u1s = loads1.pop(blk)

        # ---- v-pass (PE): v[(j,il-bcast),(st,n,p16)] = sum_i u1 ----
        v_sb = pvsb.tile([128, N * BLK], f16, tag="vsb")
        v_sb_v = v_sb[:].rearrange("P (st f) -> P st f", st=4)
        for st in range(4):
            u1_v = u1s[st][:].rearrange("P (ib f) -> P ib f", ib=4)
            v_ps = pvps.tile([128, 512], f32, tag="vps")
            for ib in range(4):
                nc.tensor.matmul(
                    v_ps[:],
                    wv_t[:],
                    u1_v[:, ib],
                    start=(ib == 0),
                    stop=(ib == 3),
                )
            nc.scalar.copy(v_sb_v[:, st], v_ps[:])
        return u1s, v_sb

    def s2(blk, u1s, v_sb):
        """c-mult (DVE) -> c-red (PE) -> softmax."""
        v_sb_v = v_sb[:].rearrange("P (st f) -> P st f", st=4)

        # ---- c-mult (2x): w1 = u1 * bcast_ib(v); st0 on Pool (it has the
        # most lead time), st1-3 on DVE ----
        w1s = []
        for st in range(4):
            u1_v = u1s[st][:].rearrange("P (ib f) -> P ib f", ib=4)
            w1 = pw1.tile([128, 4 * N * 16], f16, tag=f"w1st{st}")
            vb = (
                v_sb_v[:, st]
                .rearrange("P (o f) -> P o f", o=1)
                .broadcast_to([128, 4, N * 16])
            )
            eng = nc.gpsimd if st == 0 else nc.vector
            eng.tensor_tensor(
                w1[:].rearrange("P (ib f) -> P ib f", ib=4), u1_v, vb,
                op=OP.mult,
            )
            w1s.append(w1)

        # ---- c-red (PE): cp[(g,il), (ib,n,p4)] = 0.25*sum_j w1 ----
        # st0's g-group last: its w1 comes from the slower Pool engine
        cp = pcps.tile([128, 4 * N * P4], f32, tag="cp")
        cp_v = cp[:].rearrange("P (ib n p) -> P ib n p", ib=4, p=P4)
        g_order = list(range(4, NG)) + list(range(4))
        for i, g in enumerate(g_order):
            st, gl = g // 4, g % 4
            off = 120 - g * 8
            w1_v = w1s[st][:].rearrange(
                "P (ib n p) -> P ib n p", ib=4, p=16
            )
            nc.tensor.matmul(
                cp_v,
                wcb_t[:, off : off + 128],
                w1_v[:, :, :, gl * P4 : (gl + 1) * P4],
                start=(i == 0),
                stop=(i == NG - 1),
                skip_group_check=True,
            )

        # ---- softmax over n (no max-subtraction) ----
        c_e = pce.tile([128, 4 * N * P4], f32, tag="ce")
        nc.scalar.activation(c_e[:], cp[:], AF.Exp)
        c_e_v = c_e[:].rearrange("P (ib n p) -> P ib n p", ib=4, p=P4)
        z = pcsb.tile([128, 4 * P4], f32, tag="z")
        z_v = z[:].rearrange("P (ib p) -> P ib p", ib=4)
        for ib in range(4):
            nc.vector.tensor_reduce(
                z_v[:, ib],
                c_e_v[:, ib].rearrange("P n p -> P p n"),
                axis=mybir.AxisListType.X,
                op=OP.add,
            )
        rz = pcsb.tile([128, 4 * P4], f32, tag="rz")
        nc.vector.reciprocal(rz[:], z[:])
        rz_v = rz[:].rearrange("P (ib p) -> P ib p", ib=4)
        c_sb = pcsb2.tile([128, 4 * N * P4], f16, tag="csb")
        c_sb_v = c_sb[:].rearrange("P (ib n p) -> P ib n p", ib=4, p=P4)
        for ib in range(4):
            rz_b = (
                rz_v[:, ib]
                .rearrange("P (o p) -> P o p", o=1)
                .broadcast_to([128, N, P4])
            )
            if with_b:
                c_f = pcsb.tile([128, N * P4], f32, tag="cf")
                nc.gpsimd.tensor_tensor(
                    c_f[:].rearrange("P (n p) -> P n p", p=P4),
                    c_e_v[:, ib],
                    rz_b,
                    op=OP.mult,
                )
                bt_v = bt_t[:].rearrange("P (ib n p) -> P ib n p", ib=4, p=P4)
                nc.gpsimd.tensor_tensor(
                    c_sb_v[:, ib], c_f[:].rearrange("P (n p) -> P n p", p=P4),
                    bt_v[:, ib], op=OP.add,
                )
            else:
                nc.gpsimd.tensor_tensor(
                    c_sb_v[:, ib], c_e_v[:, ib], rz_b, op=OP.mult
                )
        return c_sb

    def s3(blk, c_sb):
        """m2 -> s-red -> squash -> store."""
        u2s = loads2.pop(blk)
        c_sb_v2 = c_sb[:].rearrange("P (ib f) -> P ib f", ib=4)  # f = (n p)
        # s_all[(g, jq, r2) parts, (jl4, n32, p4)]; j = jq*4 + jl
        s_all = psps.tile([128, 4 * N * P4], f32, tag="sall")
        s_all_v = s_all[:].rearrange("P (jl f) -> P jl f", jl=4)
        for ib in range(4):
            u2_v = u2s[ib][:].rearrange("P (j f) -> P j f", j=J)
            m2 = pm2.tile([128, J * N * P4], f16, tag="m2")
            m2_v = m2[:].rearrange("P (j f) -> P j f", j=J)
            cb = (
                c_sb_v2[:, ib]
                .rearrange("P (o f) -> P o f", o=1)
                .broadcast_to([128, J, N * P4])
            )
            nc.vector.tensor_tensor(m2_v, u2_v, cb, op=OP.mult)
            for jq in range(4):
                off = 6 - jq * 2
                nc.tensor.matmul(
                    s_all_v,
                    ws_t[:, off : off + 128],
                    m2_v[:, jq * 4 : (jq + 1) * 4],
                    start=(ib == 0 and jq == 0),
                    stop=(ib == 3 and jq == 3),
                    skip_group_check=True,
                )

        # ---- squash ----
        # ssq = s^2 (bf16 keeps fp32 range; fp16 would flush subnormals)
        ssq = psq.tile([128, 4 * N * P4], bf16, tag="ssq")
        nc.scalar.activation(ssq[:], s_all[:], AF.Square)
        ssq_v = ssq[:].rearrange("P (jl f) -> P jl f", jl=4)
        t1 = psq.tile([128, 2 * N * P4], bf16, tag="t1")
        t1_v = t1[:].rearrange("P (jl f) -> P jl f", jl=2)
        nc.gpsimd.tensor_tensor(t1_v, ssq_v[:, 0:2], ssq_v[:, 2:4], op=OP.add)
        ssq_l = psq.tile([128, N * P4], bf16, tag="ssql")
        nc.gpsimd.tensor_tensor(ssq_l[:], t1_v[:, 0], t1_v[:, 1], op=OP.add)
        # n2[(g,x8), (n,p4)] = sum_j s^2 via PE partition contraction
        n2 = pnps.tile([128, 4 * 8 * P4], f32, tag="n2")
        nc.tensor.matmul(n2[:], wn_t[:], ssq_l[:], start=True, stop=True)
        # ln(n2 + 1e-30): the bias keeps ln finite at n2==0 (out ~0 there)
        lnn = psq.tile([128, 4 * 8 * P4], f32, tag="lnn")
        nc.scalar.activation(lnn[:], n2[:], AF.Ln, bias=eps_t[:])
        # r = exp(.5 ln n2) = |s|; rn = exp(-.5 ln n2) = 1/|s|
        r_t = psq.tile([128, 4 * 8 * P4], f32, tag="r")
        nc.scalar.activation(r_t[:], lnn[:], AF.Exp, scale=0.5)
        rn_t = psq.tile([128, 4 * 8 * P4], f32, tag="rn")
        nc.scalar.activation(rn_t[:], lnn[:], AF.Exp, scale=-0.5)
        en_t = psq.tile([128, 4 * 8 * P4], f32, tag="en")
        nc.scalar.activation(en_t[:], r_t[:], AF.Exp, scale=-1.0)
        g_t = psq.tile([128, N * P4], f32, tag="g")
        nc.vector.scalar_tensor_tensor(
            g_t[:], en_t[:], 1.0, rn_t[:], op0=OP.subtract, op1=OP.mult
        )  # g = (en - 1) / r
        g_b = (
            g_t[:]
            .rearrange("P (o f) -> P o f", o=1)
            .broadcast_to([128, 4, N * P4])
        )

        # out_dev = s * g = -out_true; the host decode negates (keeps the
        # final multiply off DVE: Act copies s out of PSUM, Pool multiplies)
        s_sb = psq.tile([128, 4 * N * P4], f16, tag="ssb")
        nc.scalar.copy(s_sb[:], s_all[:])
        outt = pout.tile([128, 4 * N * P4], f16, tag="outt")
        nc.gpsimd.tensor_tensor(
            outt[:].rearrange("P (jl f) -> P jl f", jl=4),
            s_sb[:].rearrange("P (jl f) -> P jl f", jl=4),
            g_b,
            op=OP.mult,
        )

        # only the r=0 replicas carry data: 64 partitions, stride 2
        nc.sync.dma_start(o_d[blk], outt[::2, :])

    # 3-stage software pipeline: emit s1(k), s2(k-1), s3(k-2) per iteration
    # so each in-order engine queue sees work in expected-ready order (the
    # PE queue in particular becomes v(k), c-red(k-1), s-red(k-2), each of
    # whose inputs is already in flight — PE stays continuously busy and at
    # full p-state).
    p1, p2 = {}, {}
    # startup: first u1 chunk, then wv (both needed by the first v matmul),
    # then the rest — each dma occupies HWDGE ~625ns, so order matters
    ts0 = []
    for st in range(4):
        u1_first = pu1.tile([128, 4 * N * 16], f16, tag=f"u1st{st}")
        ts0.append(u1_first)
    nc.sync.dma_start(ts0[0][:], u1_d[0, 0])
    nc.sync.dma_start(wv_t[:], aps["wv"])
    for st in range(1, 4):
        nc.sync.dma_start(ts0[st][:], u1_d[0, st])
    nc.sync.dma_start(wcb_t[:], aps["wc_band"])
    nc.sync.dma_start(ws_t[:], aps["ws_band"])
    nc.sync.dma_start(wn_t[:], aps["wn"])
    loads1[0] = ts0
    # per iteration: s1(k), s3(k-2), s2(k-1) — s3's m2 DVE ops must precede
    # s2's c-mult ops in the DVE queue (PE consumes m2 first), and the out
    # store stays last on SP behind both u1 and u2 loads
    for it in range(NBLK + 2):
        if it < NBLK:
            p1[it] = s1(it)
        if it == 1:
            load2(0)
        if it >= 2:
            if it - 1 < NBLK:
                load2(it - 1)
            s3(it - 2, p2.pop(it - 2))
        if 1 <= it <= NBLK:
            p2[it - 1] = s2(it - 1, *p1.pop(it - 1))


def round_f16(x):
    return x.astype(np.float16)


def encode_u1(shard):
    """[I, N, J, pix] -> [blk, st4, (j,il)=128, (ib,n,p16)] fp16."""
    a = shard.reshape(4, 8, N, J, NBLK, 4, 16)     # ib, il, n, j, blk, st, p16
    # -> blk, st, j, il, ib, n, p16
    return np.ascontiguousarray(
        a.transpose(4, 5, 3, 1, 0, 2, 6)
    ).astype(np.float16)


def encode_u2(shard):
    """[I, N, J, pix] -> [blk, ib4, (g,il)=128, (j,n,p4)] fp16."""
    a = shard.reshape(4, 8, N, J, NBLK, NG, P4)       # ib, il, n, j, blk, g, p4
    # -> blk, ib, g, il, j, n, p4
    return np.ascontiguousarray(
        a.transpose(4, 0, 5, 1, 3, 2, 6)
    ).astype(np.float16)


def decode_out(arr):
    """[blk, 64=(g,jq), (jl4,n32,p4)] fp16 -> [N, J, pix] f32.

    j = jq*4+jl; pixel = blk*64 + g*4 + p
    """
    a = arr.astype(np.float32).reshape(NBLK, NG, 4, 4, N, P4)
    # dims: blk, g, jq, jl, n, p -> n, (jq,jl), (blk,g,p)
    # negated: the device computes s*(en-1)/r = -out
    return -np.ascontiguousarray(a.transpose(4, 2, 3, 0, 1, 5)).reshape(
        N, J, PIX
    )


_CACHE = {}


def _patch_act_tables():
    """Keep only natural_log_exp_and_others (Copy/Exp/Ln/Square): every
    function this kernel uses lives in one table, so exactly ONE
    LoadActFuncSet is emitted. Other set entries are kept (emptied) to
    preserve act_func_set_id indices."""
    if getattr(bacc, "_ant_act_tables_patched", False):
        return
    real = bacc.get_activation_tables

    def patched(module_arch):
        tabs = real(module_arch)
        keep = {"natural_log_exp_and_others"}
        return {
            name: (fns if name in keep else set()) for name, fns in tabs.items()
        }

    bacc.get_activation_tables = patched
    bacc._ant_act_tables_patched = True


def _get_program(with_b=False):
    key = with_b
    if key in _CACHE:
        return _CACHE[key]
    _patch_act_tables()
    nc = bacc.Bacc("TRN2", target_bir_lowering=False, debug=False)
    aps = {}
    aps["u1"] = nc.dram_tensor(
        "u1", [NBLK, 4, 128, 4 * N * 16], f16, kind="ExternalInput"
    ).ap()
    aps["u2"] = nc.dram_tensor(
        "u2", [NBLK, 4, 128, J * N * P4], f16, kind="ExternalInput"
    ).ap()
    wts = _build_weight_arrays()
    aps["wv"] = nc.dram_tensor("wv", [128, 128], f16, kind="ExternalInput").ap()
    aps["wc_band"] = nc.dram_tensor(
        "wc_band", [128, 248], f16, kind="ExternalInput"
    ).ap()
    aps["ws_band"] = nc.dram_tensor(
        "ws_band", [128, 134], f16, kind="ExternalInput"
    ).ap()
    aps["wn"] = nc.dram_tensor("wn", [128, 128], f16, kind="ExternalInput").ap()
    if with_b:
        aps["bt"] = nc.dram_tensor(
            "bt", [128, 4 * N * P4], f32, kind="ExternalInput"
        ).ap()
    aps["out"] = nc.dram_tensor(
        "out", [NBLK, 64, 4 * N * P4], f16, kind="ExternalOutput"
    ).ap()

    with tile.TileContext(nc) as tc:
        with ExitStack() as ctx:
            _emit(ctx, tc, aps, with_b)
    nc.compile()

    _CACHE[key] = (nc, wts)
    return _CACHE[key]


def kernel(u: np.ndarray, b: np.ndarray) -> np.ndarray:
    u = np.asarray(u, dtype=np.float32)
    b = np.asarray(b, dtype=np.float32)
    with_b = bool(np.any(b))
    nc, wts = _get_program(with_b=with_b)

    base = {
        "wv": wts["wv"].astype(np.float16),
        "wc_band": wts["wc_band"].astype(np.float16),
        "ws_band": wts["ws_band"].astype(np.float16),
        "wn": wts["wn"].astype(np.float16),
    }
    if with_b:
        base["bt"] = _b_tile_array(b)
    in_maps = []
    for c in range(NCORES):
        bb = c // 2
        h0 = 16 * (c % 2)
        shard = u[bb, :, :, :, h0 : h0 + 16, :].reshape(I, N, J, PIX)
        m = dict(base)
        m["u1"] = encode_u1(shard)
        m["u2"] = encode_u2(shard)
        in_maps.append(m)

    res = run_bass_kernel_spmd(nc, in_maps, core_ids=list(range(NCORES)))
    out = np.zeros((B, N, J, H, W), np.float32)
    for c in range(NCORES):
        bb = c // 2
        h0 = 16 * (c % 2)
        out[bb, :, :, h0 : h0 + 16, :] = decode_out(res.results[c]["out"]).reshape(
            N, J, 16, W
        )
    return out
